# revision 7
# baseline (speedup 1.0000x reference)
"""Trainium2 Bass kernel for nn_MultiHeadAttention_6090263626512.

Full inputs -> full outputs; internally sharded over 8 NeuronCores:
batch (2) x head-groups (4 heads each) -> 8 shards (data + tensor parallel).
Each core computes Q/K/V projections for its 4 heads, full-score attention,
its partial output projection, then a ReduceScatter over the 4 cores of its
batch group combines the output projection partials.

Per-core device program (all matmuls in fp32r = TRN2 fast fp32 mode):
  phase 0: load W/X, build X^T via PE transposes
  phase 1: Q^T/K^T ([j,n] layout) and V ([n,v] layout) projections,
           biases folded into the PSUM->SBUF moves
  phase 2: per (q-group, head): scores -> exp (+row sums) -> normalize ->
           DMA alpha out; PE-transpose normalized alpha -> alpha^T;
           Z^T accumulation (V^T @ alpha^T)
  phase 3: output projection per q-group; DMA partial out
  phase 4: ReduceScatter(add) over the 4-core batch group

Outputs per core: alpha shard [4, 2048, 2048] f32, out shard [512, 1024] f32.
"""

import numpy as np

B, N, D = 2, 2048, 1024
H, DK = 16, 64
HL = 4              # heads per core
J = HL * DK         # 256 local projection width
P = 128
NT = N // P         # 16 q/k/n tiles
KC = D // P         # 8 contraction chunks of the model dim
NCH = N // 512      # 4 512-wide chunks of a row
GQ = 4              # q-tiles per q-group
NG = NT // GQ       # 4 q-groups

_CACHE = {}


def _build():
    from concourse import bacc, mybir, tile, masks

    f32 = mybir.dt.float32
    f32r = mybir.dt.float32r
    Exp = mybir.ActivationFunctionType.Exp
    mult = mybir.AluOpType.mult
    add = mybir.AluOpType.add

    nc = bacc.Bacc("TRN2", target_bir_lowering=False, debug=False, num_devices=8)

    Xp = nc.declare_dram_parameter("x", [N, D], f32r, isOutput=False)
    Wqp = nc.declare_dram_parameter("wq", [D, J], f32r, isOutput=False)
    Wkp = nc.declare_dram_parameter("wk", [D, J], f32r, isOutput=False)
    Wvp = nc.declare_dram_parameter("wv", [D, J], f32r, isOutput=False)
    Wop = nc.declare_dram_parameter("wo", [J, D], f32r, isOutput=False)
    bqp = nc.declare_dram_parameter("bq", [J], f32, isOutput=False)
    bkp = nc.declare_dram_parameter("bk", [J], f32, isOutput=False)
    bvp = nc.declare_dram_parameter("bv", [J], f32, isOutput=False)
    identp = nc.declare_dram_parameter("ident", [P, P], f32r, isOutput=False)
    alpha_o = nc.declare_dram_parameter("alpha", [HL, N, N], f32r, isOutput=True)
    out_o = nc.declare_dram_parameter("outp", [N // 4, D], f32, isOutput=True)

    with tile.TileContext(nc) as tc:
        with tc.tile_pool(name="dram", bufs=1, space="DRAM") as dram, \
             tc.tile_pool(name="const", bufs=1) as cpool, \
             tc.tile_pool(name="wpool", bufs=1) as wpool, \
             tc.tile_pool(name="qkv", bufs=1) as qkv, \
             tc.tile_pool(name="ztnp", bufs=1) as ztnp:

            po = dram.tile([N, D], f32)
            rs_tiles = [dram.tile([P, D], f32, name=f"rs{g}", tag=f"rs{g}") for g in range(NG)]

            ident = cpool.tile([P, P], f32r)
            nc.sync.dma_start(out=ident[:], in_=identp[:])
            bq_sb = cpool.tile([P, 2], f32)
            bk_sb = cpool.tile([P, 2], f32)
            bv_bc = cpool.tile([P, J], f32)
            bv_row = cpool.tile([1, J], f32)
            ones_row = cpool.tile([1, P], f32)
            nc.gpsimd.memset(ones_row[:], 1.0)
            for jt in range(2):
                nc.sync.dma_start(out=bq_sb[:, jt:jt + 1], in_=bqp[jt * P:(jt + 1) * P])
                nc.sync.dma_start(out=bk_sb[:, jt:jt + 1], in_=bkp[jt * P:(jt + 1) * P])
            nc.sync.dma_start(out=bv_row[:], in_=bvp[:])

            wq_sb = wpool.tile([P, KC, J], f32r)
            wk_sb = wpool.tile([P, KC, J], f32r)
            wv_sb = wpool.tile([P, KC, J], f32r)
            wo_sb = wpool.tile([P, 2, D], f32r)
            for kc in range(KC):
                nc.sync.dma_start(out=wq_sb[:, kc, :], in_=Wqp[kc * P:(kc + 1) * P, :])
                nc.sync.dma_start(out=wk_sb[:, kc, :], in_=Wkp[kc * P:(kc + 1) * P, :])
                nc.sync.dma_start(out=wv_sb[:, kc, :], in_=Wvp[kc * P:(kc + 1) * P, :])
            for jt in range(2):
                nc.sync.dma_start(out=wo_sb[:, jt, :], in_=Wop[jt * P:(jt + 1) * P, :])

            qt_sb = qkv.tile([P, 2, N], f32r)
            kt_sb = qkv.tile([P, 2, N], f32r)
            v_sb = qkv.tile([P, NT, J], f32r)
            ztn_sb = ztnp.tile([P, 2, N], f32r)

            # ---- phase 0: X^T ----
            with tc.tile_pool(name="xtp", bufs=1) as xtp, \
                 tc.tile_pool(name="xnat", bufs=3) as xnat, \
                 tc.tile_pool(name="ps01", bufs=4, space="PSUM") as ps01:
                xt_sb = xtp.tile([P, KC, N], f32r)
                for nt in range(NT):
                    xn = xnat.tile([P, D], f32r)
                    nc.sync.dma_start(out=xn[:], in_=Xp[nt * P:(nt + 1) * P, :])
                    for half in range(2):
                        pt = ps01.tile([P, 4, P], f32r, tag="s")
                        for q in range(4):
                            kc = half * 4 + q
                            nc.tensor.transpose(
                                pt[:, q, :], xn[:, kc * P:(kc + 1) * P], ident[:]
                            )
                        nc.scalar.copy(
                            xt_sb[:, half * 4:(half + 1) * 4, nt * P:(nt + 1) * P],
                            pt[:],
                        )

                # ---- phase 1: projections ----
                for jt in range(2):
                    for ch in range(NCH):
                        pq = ps01.tile([P, 512], f32, tag="s")
                        pk = ps01.tile([P, 512], f32, tag="s")
                        for kc in range(KC):
                            nc.tensor.matmul(
                                pq[:], wq_sb[:, kc, jt * P:(jt + 1) * P],
                                xt_sb[:, kc, ch * 512:(ch + 1) * 512],
                                start=(kc == 0), stop=(kc == KC - 1),
                            )
                        for kc in range(KC):
                            nc.tensor.matmul(
                                pk[:], wk_sb[:, kc, jt * P:(jt + 1) * P],
                                xt_sb[:, kc, ch * 512:(ch + 1) * 512],
                                start=(kc == 0), stop=(kc == KC - 1),
                            )
                        nc.vector.tensor_scalar(
                            out=qt_sb[:, jt, ch * 512:(ch + 1) * 512], in0=pq[:],
                            scalar1=bq_sb[:, jt:jt + 1], scalar2=None, op0=add,
                        )
                        nc.vector.tensor_scalar(
                            out=kt_sb[:, jt, ch * 512:(ch + 1) * 512], in0=pk[:],
                            scalar1=bk_sb[:, jt:jt + 1], scalar2=None, op0=add,
                        )

                # bv broadcast [128, J] via K=1 f32 matmul of ones^T x bv_row
                pbv = ps01.tile([P, J], f32, tag="s")
                nc.tensor.matmul(pbv[:], ones_row[:], bv_row[:], start=True, stop=True)
                nc.vector.tensor_copy(bv_bc[:], pbv[:])

                for nt in range(NT):
                    pv = ps01.tile([P, J], f32, tag="s")
                    for kc in range(KC):
                        nc.tensor.matmul(
                            pv[:], xt_sb[:, kc, nt * P:(nt + 1) * P],
                            wv_sb[:, kc, :],
                            start=(kc == 0), stop=(kc == KC - 1),
                        )
                    nc.vector.tensor_tensor(
                        out=v_sb[:, nt, :], in0=pv[:], in1=bv_bc[:], op=add
                    )

            # ---- phase 2+3: attention per q-group ----
            with tc.tile_pool(name="alpha_u", bufs=2) as aup, \
                 tc.tile_pool(name="alpha_n", bufs=6) as anp, \
                 tc.tile_pool(name="alphaT", bufs=4) as atp, \
                 tc.tile_pool(name="sm", bufs=8) as smp, \
                 tc.tile_pool(name="outsb", bufs=2) as outp, \
                 tc.tile_pool(name="ps_a", bufs=1, space="PSUM") as ps_a, \
                 tc.tile_pool(name="ps_t", bufs=2, space="PSUM") as ps_t, \
                 tc.tile_pool(name="ps_z", bufs=2, space="PSUM") as ps_z:

                for g in range(NG):
                    for h in range(HL):
                        jt, jr = h // 2, (h % 2) * DK
                        an_tiles = []
                        for qq in range(GQ):
                            qi = g * GQ + qq
                            pa = ps_a.tile([P, N], f32)
                            for ch in range(NCH):
                                nc.tensor.matmul(
                                    pa[:, ch * 512:(ch + 1) * 512],
                                    qt_sb[jr:jr + DK, jt, qi * P:(qi + 1) * P],
                                    kt_sb[jr:jr + DK, jt, ch * 512:(ch + 1) * 512],
                                    start=True, stop=True,
                                )
                            ex = aup.tile([P, N], f32)
                            sums = smp.tile([P, 1], f32, tag="sums")
                            nc.scalar.activation(ex[:], pa[:], Exp,
                                                 bias=0.0, scale=0.125,
                                                 accum_out=sums[:])
                            rec = smp.tile([P, 1], f32, tag="rec")
                            nc.vector.reciprocal(rec[:], sums[:])
                            an = anp.tile([P, N], f32r)
                            nc.vector.tensor_scalar(out=an[:], in0=ex[:],
                                                    scalar1=rec[:], scalar2=None,
                                                    op0=mult)
                            nc.sync.dma_start(
                                out=alpha_o[h, qi * P:(qi + 1) * P, :], in_=an[:]
                            )
                            an_tiles.append(an)

                        ztp = ps_z.tile([DK, 512], f32)
                        for kt in range(NT):
                            pt = ps_t.tile([P, 512], f32r, tag="t")
                            for qq in range(GQ):
                                nc.tensor.transpose(
                                    pt[:, qq * P:(qq + 1) * P],
                                    an_tiles[qq][:, kt * P:(kt + 1) * P],
                                    ident[:],
                                )
                            at = atp.tile([P, 512], f32r)
                            if kt % 2 == 0:
                                nc.vector.tensor_copy(at[:], pt[:])
                            else:
                                nc.scalar.copy(at[:], pt[:])
                            nc.tensor.matmul(
                                ztp[:], v_sb[:, kt, h * DK:(h + 1) * DK], at[:],
                                start=(kt == 0), stop=(kt == NT - 1),
                            )
                        nc.vector.tensor_copy(
                            ztn_sb[jr:jr + DK, jt, g * 512:(g + 1) * 512], ztp[:]
                        )

                    # ---- phase 3: output projection for this q-group ----
                    for qq in range(GQ):
                        qi = g * GQ + qq
                        ob = outp.tile([P, D], f32)
                        for mc in range(2):
                            pp = ps_t.tile([P, 512], f32, tag="t")
                            for jt2 in range(2):
                                nc.tensor.matmul(
                                    pp[:], ztn_sb[:, jt2, qi * P:(qi + 1) * P],
                                    wo_sb[:, jt2, mc * 512:(mc + 1) * 512],
                                    start=(jt2 == 0), stop=(jt2 == 1),
                                )
                            nc.vector.tensor_copy(ob[:, mc * 512:(mc + 1) * 512], pp[:])
                        nc.sync.dma_start(out=po[qi * P:(qi + 1) * P, :], in_=ob[:])
                    nc.gpsimd.collective_compute(
                        "ReduceScatter", add,
                        replica_groups=[[0, 1, 2, 3], [4, 5, 6, 7]],
                        ins=[po[g * 512:(g + 1) * 512, :]], outs=[rs_tiles[g][:]],
                    )
                    nc.sync.dma_start(
                        out=out_o[g * P:(g + 1) * P, :], in_=rs_tiles[g][:]
                    )


    nc.compile()
    return nc


def _get_nc():
    if "nc" not in _CACHE:
        _CACHE["nc"] = _build()
    return _CACHE["nc"]


def _shard_inputs(X, Wq, bq, Wk, bk, Wv, bv, Wo, bo):
    in_maps = []
    for c in range(8):
        b, hg = c // 4, c % 4
        js = slice(hg * J, (hg + 1) * J)
        in_maps.append({
            "x": np.ascontiguousarray(X[b]),
            "wq": np.ascontiguousarray(Wq[:, js]),
            "wk": np.ascontiguousarray(Wk[:, js]),
            "wv": np.ascontiguousarray(Wv[:, js]),
            "wo": np.ascontiguousarray(Wo[js, :]),
            "bq": np.ascontiguousarray(bq[js]),
            "bk": np.ascontiguousarray(bk[js]),
            "bv": np.ascontiguousarray(bv[js]),
            "ident": np.eye(P, dtype=np.float32),
        })
    return in_maps


def _assemble(results, bo):
    out = np.empty((B, N, D), dtype=np.float32)
    alpha = np.empty((B, H, N, N), dtype=np.float32)
    for c in range(8):
        b, hg = c // 4, c % 4
        alpha[b, hg * HL:(hg + 1) * HL] = results[c]["alpha"]
        shard = results[c]["outp"]
        for g in range(NG):
            out[b, g * 512 + hg * P:g * 512 + (hg + 1) * P] = shard[g * P:(g + 1) * P]
    out += bo.astype(np.float32)
    return out, alpha


def _run(in_maps, trace=False):
    from concourse.bass_utils import run_bass_kernel_spmd
    nc = _get_nc()
    return run_bass_kernel_spmd(nc, in_maps, list(range(8)), trace=trace)


def kernel(X, Wq, bq, Wk, bk, Wv, bv, Wo, bo):
    args = [np.asarray(a, dtype=np.float32) for a in
            (X, Wq, bq, Wk, bk, Wv, bv, Wo, bo)]
    in_maps = _shard_inputs(*args)
    res = _run(in_maps, trace=False)
    return _assemble(res.results, args[-1])


# revision 17
# speedup vs baseline: 1.1201x; 1.1201x over previous
"""Trainium2 Bass kernel for nn_MultiHeadAttention_6090263626512.

Full inputs -> full outputs; internally sharded over 8 NeuronCores:
batch (2) x head-groups (4 heads each) -> 8 shards (data + tensor parallel).
Each core computes Q/K/V projections for its 4 heads, full-score attention,
its partial output projection, then a ReduceScatter over the 4 cores of its
batch group combines the output projection partials.

Per-core device program (all matmuls in fp32r = TRN2 fast fp32 mode):
  phase 0: load W/X, build X^T via PE transposes
  phase 1: Q^T/K^T ([j,n] layout) and V ([n,v] layout) projections,
           biases folded into the PSUM->SBUF moves
  phase 2: per (q-group, head): scores -> exp (+row sums) -> normalize ->
           DMA alpha out; PE-transpose normalized alpha -> alpha^T;
           Z^T accumulation (V^T @ alpha^T)
  phase 3: output projection per q-group; DMA partial out
  phase 4: ReduceScatter(add) over the 4-core batch group

Outputs per core: alpha shard [4, 2048, 2048] f32, out shard [512, 1024] f32.
"""

import numpy as np

B, N, D = 2, 2048, 1024
H, DK = 16, 64
HL = 4              # heads per core
J = HL * DK         # 256 local projection width
P = 128
NT = N // P         # 16 q/k/n tiles
KC = D // P         # 8 contraction chunks of the model dim
NCH = N // 512      # 4 512-wide chunks of a row
GQ = 4              # q-tiles per q-group
NG = NT // GQ       # 4 q-groups

_CACHE = {}


def _build():
    from concourse import bacc, mybir, tile, masks

    f32 = mybir.dt.float32
    f32r = mybir.dt.float32r
    Exp = mybir.ActivationFunctionType.Exp
    mult = mybir.AluOpType.mult
    add = mybir.AluOpType.add

    nc = bacc.Bacc("TRN2", target_bir_lowering=False, debug=False, num_devices=8)

    Xp = nc.declare_dram_parameter("x", [N, D], f32, isOutput=False)
    Wqp = nc.declare_dram_parameter("wq", [D, J], f32r, isOutput=False)
    Wkp = nc.declare_dram_parameter("wk", [D, J], f32r, isOutput=False)
    Wvp = nc.declare_dram_parameter("wv", [D, J], f32r, isOutput=False)
    Wop = nc.declare_dram_parameter("wo", [J, D], f32r, isOutput=False)
    bqp = nc.declare_dram_parameter("bq", [J], f32, isOutput=False)
    bkp = nc.declare_dram_parameter("bk", [J], f32, isOutput=False)
    bvp = nc.declare_dram_parameter("bv", [J], f32, isOutput=False)
    alpha_o = nc.declare_dram_parameter("alpha", [HL, N, N], f32, isOutput=True)
    out_o = nc.declare_dram_parameter("outp", [N // 4, D], f32, isOutput=True)

    with tile.TileContext(nc) as tc:
        with tc.tile_pool(name="dram", bufs=1, space="DRAM") as dram, \
             tc.tile_pool(name="const", bufs=1) as cpool, \
             tc.tile_pool(name="wpool", bufs=1) as wpool, \
             tc.tile_pool(name="qkv", bufs=1) as qkv, \
             tc.tile_pool(name="ztnp", bufs=1) as ztnp:

            po = dram.tile([N, D], f32)
            rs_tiles = [dram.tile([P, D], f32, name=f"rs{g}", tag=f"rs{g}") for g in range(NG)]

            ident = cpool.tile([P, P], f32)
            masks.make_identity(nc, ident[:])
            bq_sb = cpool.tile([P, 2], f32)
            bk_sb = cpool.tile([P, 2], f32)
            bv_bc = cpool.tile([P, J], f32)
            bv_row = cpool.tile([1, J], f32)
            ones_row = cpool.tile([1, P], f32)
            nc.gpsimd.memset(ones_row[:], 1.0)
            for jt in range(2):
                nc.sync.dma_start(out=bq_sb[:, jt:jt + 1], in_=bqp[jt * P:(jt + 1) * P])
                nc.sync.dma_start(out=bk_sb[:, jt:jt + 1], in_=bkp[jt * P:(jt + 1) * P])
            nc.sync.dma_start(out=bv_row[:], in_=bvp[:])

            wq_sb = wpool.tile([P, KC, J], f32r)
            wk_sb = wpool.tile([P, KC, J], f32r)
            wv_sb = wpool.tile([P, KC, J], f32r)
            wo_sb = wpool.tile([P, 2, D], f32r)

            qt_sb = qkv.tile([P, 2, N], f32r)
            kt_sb = qkv.tile([P, 2, N], f32r)
            v_sb = qkv.tile([P, NT, J], f32r)
            ztn_sb = ztnp.tile([P, 2, N], f32r)

            # ---- phase 0: X^T ----
            with tc.tile_pool(name="xtp", bufs=1) as xtp, \
                 tc.tile_pool(name="xnat", bufs=4) as xnat, \
                 tc.tile_pool(name="ps01", bufs=6, space="PSUM") as ps01:
                xt_sb = xtp.tile([P, KC, N], f32r)
                for kc in range(KC):
                    nc.sync.dma_start(out=wq_sb[:, kc, :], in_=Wqp[kc * P:(kc + 1) * P, :])
                    nc.sync.dma_start(out=wk_sb[:, kc, :], in_=Wkp[kc * P:(kc + 1) * P, :])
                    nc.sync.dma_start(out=wv_sb[:, kc, :], in_=Wvp[kc * P:(kc + 1) * P, :])
                for jt in range(2):
                    nc.sync.dma_start(out=wo_sb[:, jt, :], in_=Wop[jt * P:(jt + 1) * P, :])
                for nt in range(NT):
                    xn = xnat.tile([P, D], f32)
                    nc.sync.dma_start(out=xn[:], in_=Xp[nt * P:(nt + 1) * P, :])
                    for half in range(2):
                        pt = ps01.tile([P, 4, P], f32, tag="s")
                        for q in range(4):
                            kc = half * 4 + q
                            nc.tensor.transpose(
                                pt[:, q, :], xn[:, kc * P:(kc + 1) * P], ident[:]
                            )
                        eng = nc.scalar if (nt + half) % 2 else nc.vector
                        (eng.copy if (nt + half) % 2 else eng.tensor_copy)(
                            xt_sb[:, half * 4:(half + 1) * 4, nt * P:(nt + 1) * P],
                            pt[:],
                        )

                # ---- phase 1: projections ----
                for jt in range(2):
                    for ch in range(NCH):
                        pq = ps01.tile([P, 512], f32, tag="s")
                        pk = ps01.tile([P, 512], f32, tag="s")
                        for kc in range(KC):
                            nc.tensor.matmul(
                                pq[:], wq_sb[:, kc, jt * P:(jt + 1) * P],
                                xt_sb[:, kc, ch * 512:(ch + 1) * 512],
                                start=(kc == 0), stop=(kc == KC - 1),
                            )
                        for kc in range(KC):
                            nc.tensor.matmul(
                                pk[:], wk_sb[:, kc, jt * P:(jt + 1) * P],
                                xt_sb[:, kc, ch * 512:(ch + 1) * 512],
                                start=(kc == 0), stop=(kc == KC - 1),
                            )
                        nc.vector.tensor_scalar(
                            out=qt_sb[:, jt, ch * 512:(ch + 1) * 512], in0=pq[:],
                            scalar1=bq_sb[:, jt:jt + 1], scalar2=None, op0=add,
                        )
                        nc.vector.tensor_scalar(
                            out=kt_sb[:, jt, ch * 512:(ch + 1) * 512], in0=pk[:],
                            scalar1=bk_sb[:, jt:jt + 1], scalar2=None, op0=add,
                        )

                # bv broadcast [128, J] via K=1 f32 matmul of ones^T x bv_row
                pbv = ps01.tile([P, J], f32, tag="s")
                nc.tensor.matmul(pbv[:], ones_row[:], bv_row[:], start=True, stop=True)
                nc.vector.tensor_copy(bv_bc[:], pbv[:])

                for nt in range(NT):
                    pv = ps01.tile([P, J], f32, tag="s")
                    for kc in range(KC):
                        nc.tensor.matmul(
                            pv[:], xt_sb[:, kc, nt * P:(nt + 1) * P],
                            wv_sb[:, kc, :],
                            start=(kc == 0), stop=(kc == KC - 1),
                        )
                    nc.vector.tensor_tensor(
                        out=v_sb[:, nt, :], in0=pv[:], in1=bv_bc[:], op=add
                    )

            # ---- phase 2+3: attention per q-group ----
            with tc.tile_pool(name="alpha_u", bufs=3) as aup, \
                 tc.tile_pool(name="alpha_n", bufs=7) as anp, \
                 tc.tile_pool(name="alphaT", bufs=4) as atp, \
                 tc.tile_pool(name="sm", bufs=8) as smp, \
                 tc.tile_pool(name="outsb", bufs=2) as outp, \
                 tc.tile_pool(name="ps_a", bufs=1, space="PSUM") as ps_a, \
                 tc.tile_pool(name="ps_t", bufs=2, space="PSUM") as ps_t, \
                 tc.tile_pool(name="ps_z", bufs=2, space="PSUM") as ps_z:

                for g in range(NG):
                    for h in range(HL):
                        jt, jr = h // 2, (h % 2) * DK
                        an_tiles = []
                        for qq in range(GQ):
                            qi = g * GQ + qq
                            pa = ps_a.tile([P, N], f32)
                            for ch in range(NCH):
                                nc.tensor.matmul(
                                    pa[:, ch * 512:(ch + 1) * 512],
                                    qt_sb[jr:jr + DK, jt, qi * P:(qi + 1) * P],
                                    kt_sb[jr:jr + DK, jt, ch * 512:(ch + 1) * 512],
                                    start=True, stop=True,
                                )
                            ex = aup.tile([P, N], f32)
                            sums = smp.tile([P, 1], f32, tag="sums")
                            nc.scalar.activation(ex[:], pa[:], Exp,
                                                 bias=0.0, scale=0.125,
                                                 accum_out=sums[:])
                            rec = smp.tile([P, 1], f32, tag="rec")
                            nc.vector.reciprocal(rec[:], sums[:])
                            an = anp.tile([P, N], f32)
                            nc.vector.tensor_scalar(out=an[:], in0=ex[:],
                                                    scalar1=rec[:], scalar2=None,
                                                    op0=mult)
                            nc.sync.dma_start(
                                out=alpha_o[h, qi * P:(qi + 1) * P, :], in_=an[:]
                            )
                            an_tiles.append(an)

                        ztp = ps_z.tile([DK, 512], f32)
                        for kt in range(NT):
                            pt = ps_t.tile([P, 512], f32, tag="t")
                            for qq in range(GQ):
                                nc.tensor.transpose(
                                    pt[:, qq * P:(qq + 1) * P],
                                    an_tiles[qq][:, kt * P:(kt + 1) * P],
                                    ident[:],
                                )
                            at = atp.tile([P, 512], f32r)
                            if kt % 2 == 0:
                                nc.vector.tensor_copy(at[:], pt[:])
                            else:
                                nc.scalar.copy(at[:], pt[:])
                            nc.tensor.matmul(
                                ztp[:], v_sb[:, kt, h * DK:(h + 1) * DK], at[:],
                                start=(kt == 0), stop=(kt == NT - 1),
                            )
                        nc.vector.tensor_copy(
                            ztn_sb[jr:jr + DK, jt, g * 512:(g + 1) * 512], ztp[:]
                        )

                    # ---- phase 3: output projection for this q-group ----
                    for qq in range(GQ):
                        qi = g * GQ + qq
                        ob = outp.tile([P, D], f32)
                        for mc in range(2):
                            pp = ps_t.tile([P, 512], f32, tag="t")
                            for jt2 in range(2):
                                nc.tensor.matmul(
                                    pp[:], ztn_sb[:, jt2, qi * P:(qi + 1) * P],
                                    wo_sb[:, jt2, mc * 512:(mc + 1) * 512],
                                    start=(jt2 == 0), stop=(jt2 == 1),
                                )
                            nc.vector.tensor_copy(ob[:, mc * 512:(mc + 1) * 512], pp[:])
                        nc.sync.dma_start(out=po[qi * P:(qi + 1) * P, :], in_=ob[:])
                    nc.gpsimd.collective_compute(
                        "ReduceScatter", add,
                        replica_groups=[[0, 1, 2, 3], [4, 5, 6, 7]],
                        ins=[po[g * 512:(g + 1) * 512, :]], outs=[rs_tiles[g][:]],
                    )
                    nc.sync.dma_start(
                        out=out_o[g * P:(g + 1) * P, :], in_=rs_tiles[g][:]
                    )


    nc.compile()
    return nc


def _get_nc():
    if "nc" not in _CACHE:
        _CACHE["nc"] = _build()
    return _CACHE["nc"]


def _shard_inputs(X, Wq, bq, Wk, bk, Wv, bv, Wo, bo):
    in_maps = []
    for c in range(8):
        b, hg = c // 4, c % 4
        js = slice(hg * J, (hg + 1) * J)
        in_maps.append({
            "x": np.ascontiguousarray(X[b]),
            "wq": np.ascontiguousarray(Wq[:, js]),
            "wk": np.ascontiguousarray(Wk[:, js]),
            "wv": np.ascontiguousarray(Wv[:, js]),
            "wo": np.ascontiguousarray(Wo[js, :]),
            "bq": np.ascontiguousarray(bq[js]),
            "bk": np.ascontiguousarray(bk[js]),
            "bv": np.ascontiguousarray(bv[js]),
        })
    return in_maps


def _assemble(results, bo):
    out = np.empty((B, N, D), dtype=np.float32)
    alpha = np.empty((B, H, N, N), dtype=np.float32)
    for c in range(8):
        b, hg = c // 4, c % 4
        alpha[b, hg * HL:(hg + 1) * HL] = results[c]["alpha"]
        shard = results[c]["outp"]
        for g in range(NG):
            out[b, g * 512 + hg * P:g * 512 + (hg + 1) * P] = shard[g * P:(g + 1) * P]
    out += bo.astype(np.float32)
    return out, alpha


def _run(in_maps, trace=False):
    from concourse.bass_utils import run_bass_kernel_spmd
    nc = _get_nc()
    return run_bass_kernel_spmd(nc, in_maps, list(range(8)), trace=trace)


def kernel(X, Wq, bq, Wk, bk, Wv, bv, Wo, bo):
    args = [np.asarray(a, dtype=np.float32) for a in
            (X, Wq, bq, Wk, bk, Wv, bv, Wo, bo)]
    in_maps = _shard_inputs(*args)
    res = _run(in_maps, trace=False)
    return _assemble(res.results, args[-1])


# revision 19
# speedup vs baseline: 1.1799x; 1.0533x over previous
"""Trainium2 Bass kernel for nn_MultiHeadAttention_6090263626512.

Full inputs -> full outputs; internally sharded over 8 NeuronCores:
batch (2) x head-groups (4 heads each) -> 8 shards (data + tensor parallel).
Each core computes Q/K/V projections for its 4 heads, full-score attention,
its partial output projection, then a ReduceScatter over the 4 cores of its
batch group combines the output projection partials.

Per-core device program (all matmuls in fp32r = TRN2 fast fp32 mode):
  phase 0: load W/X, build X^T via PE transposes
  phase 1: Q^T/K^T ([j,n] layout) and V ([n,v] layout) projections,
           biases folded into the PSUM->SBUF moves
  phase 2: per (q-group, head): scores -> exp (+row sums) -> normalize ->
           DMA alpha out; PE-transpose normalized alpha -> alpha^T;
           Z^T accumulation (V^T @ alpha^T)
  phase 3: output projection per q-group; DMA partial out
  phase 4: ReduceScatter(add) over the 4-core batch group

Outputs per core: alpha shard [4, 2048, 2048] f32, out shard [512, 1024] f32.
"""

import numpy as np

B, N, D = 2, 2048, 1024
H, DK = 16, 64
HL = 4              # heads per core
J = HL * DK         # 256 local projection width
P = 128
NT = N // P         # 16 q/k/n tiles
KC = D // P         # 8 contraction chunks of the model dim
NCH = N // 512      # 4 512-wide chunks of a row
GQ = 4              # q-tiles per q-group
NG = NT // GQ       # 4 q-groups

_CACHE = {}


def _build():
    from concourse import bacc, mybir, tile, masks

    f32 = mybir.dt.float32
    f32r = mybir.dt.float32r
    Exp = mybir.ActivationFunctionType.Exp
    mult = mybir.AluOpType.mult
    add = mybir.AluOpType.add

    nc = bacc.Bacc("TRN2", target_bir_lowering=False, debug=False, num_devices=8)

    Xp = nc.declare_dram_parameter("x", [N, D], f32, isOutput=False)
    Wqp = nc.declare_dram_parameter("wq", [D, J], f32r, isOutput=False)
    Wkp = nc.declare_dram_parameter("wk", [D, J], f32r, isOutput=False)
    Wvp = nc.declare_dram_parameter("wv", [D, J], f32r, isOutput=False)
    Wop = nc.declare_dram_parameter("wo", [J, D], f32r, isOutput=False)
    bqp = nc.declare_dram_parameter("bq", [J], f32, isOutput=False)
    bkp = nc.declare_dram_parameter("bk", [J], f32, isOutput=False)
    bvp = nc.declare_dram_parameter("bv", [J], f32, isOutput=False)
    alpha_o = nc.declare_dram_parameter("alpha", [HL, N, N], f32, isOutput=True)
    out_o = nc.declare_dram_parameter("outp", [N // 4, D], f32, isOutput=True)

    with tile.TileContext(nc) as tc:
        with tc.tile_pool(name="dram", bufs=1, space="DRAM") as dram, \
             tc.tile_pool(name="const", bufs=1) as cpool, \
             tc.tile_pool(name="wpool", bufs=1) as wpool, \
             tc.tile_pool(name="qkv", bufs=1) as qkv, \
             tc.tile_pool(name="ztnp", bufs=1) as ztnp:

            po = dram.tile([N, D], f32)
            rs_tiles = [dram.tile([P, D], f32, name=f"rs{g}", tag=f"rs{g}") for g in range(NG)]

            ident = cpool.tile([P, P], f32)
            masks.make_identity(nc, ident[:])
            bq_sb = cpool.tile([P, 2], f32)
            bk_sb = cpool.tile([P, 2], f32)
            bv_bc = cpool.tile([P, J], f32)
            bv_row = cpool.tile([1, J], f32)
            ones_row = cpool.tile([1, P], f32)
            nc.gpsimd.memset(ones_row[:], 1.0)
            for jt in range(2):
                nc.sync.dma_start(out=bq_sb[:, jt:jt + 1], in_=bqp[jt * P:(jt + 1) * P])
                nc.sync.dma_start(out=bk_sb[:, jt:jt + 1], in_=bkp[jt * P:(jt + 1) * P])
            nc.sync.dma_start(out=bv_row[:], in_=bvp[:])

            wq_sb = wpool.tile([P, KC, J], f32r)
            wk_sb = wpool.tile([P, KC, J], f32r)
            wv_sb = wpool.tile([P, KC, J], f32r)
            wo_sb = wpool.tile([P, 2, D], f32r)

            qt_sb = qkv.tile([P, 2, N], f32r)
            kt_sb = qkv.tile([P, 2, N], f32r)
            v_sb = qkv.tile([P, NT, J], f32r)
            ztn_sb = ztnp.tile([P, 2, N], f32r)

            # ---- phase 0: X^T ----
            with tc.tile_pool(name="xtp", bufs=1) as xtp, \
                 tc.tile_pool(name="xnat", bufs=4) as xnat, \
                 tc.tile_pool(name="ps01", bufs=6, space="PSUM") as ps01:
                xt_sb = xtp.tile([P, KC, N], f32r)
                for kc in range(KC):
                    nc.sync.dma_start(out=wq_sb[:, kc, :], in_=Wqp[kc * P:(kc + 1) * P, :])
                    nc.sync.dma_start(out=wk_sb[:, kc, :], in_=Wkp[kc * P:(kc + 1) * P, :])
                    nc.sync.dma_start(out=wv_sb[:, kc, :], in_=Wvp[kc * P:(kc + 1) * P, :])
                for jt in range(2):
                    nc.sync.dma_start(out=wo_sb[:, jt, :], in_=Wop[jt * P:(jt + 1) * P, :])
                for nt in range(NT):
                    xn = xnat.tile([P, D], f32)
                    nc.sync.dma_start(out=xn[:], in_=Xp[nt * P:(nt + 1) * P, :])
                    for half in range(2):
                        pt = ps01.tile([P, 4, P], f32, tag="s")
                        for q in range(4):
                            kc = half * 4 + q
                            nc.tensor.transpose(
                                pt[:, q, :], xn[:, kc * P:(kc + 1) * P], ident[:]
                            )
                        eng = nc.scalar if (nt + half) % 2 else nc.vector
                        (eng.copy if (nt + half) % 2 else eng.tensor_copy)(
                            xt_sb[:, half * 4:(half + 1) * 4, nt * P:(nt + 1) * P],
                            pt[:],
                        )

                # ---- phase 1: projections ----
                for jt in range(2):
                    for ch in range(NCH):
                        pq = ps01.tile([P, 512], f32, tag="s")
                        pk = ps01.tile([P, 512], f32, tag="s")
                        for kc in range(KC):
                            nc.tensor.matmul(
                                pq[:], wq_sb[:, kc, jt * P:(jt + 1) * P],
                                xt_sb[:, kc, ch * 512:(ch + 1) * 512],
                                start=(kc == 0), stop=(kc == KC - 1),
                            )
                        for kc in range(KC):
                            nc.tensor.matmul(
                                pk[:], wk_sb[:, kc, jt * P:(jt + 1) * P],
                                xt_sb[:, kc, ch * 512:(ch + 1) * 512],
                                start=(kc == 0), stop=(kc == KC - 1),
                            )
                        nc.vector.tensor_scalar(
                            out=qt_sb[:, jt, ch * 512:(ch + 1) * 512], in0=pq[:],
                            scalar1=bq_sb[:, jt:jt + 1], scalar2=None, op0=add,
                        )
                        nc.vector.tensor_scalar(
                            out=kt_sb[:, jt, ch * 512:(ch + 1) * 512], in0=pk[:],
                            scalar1=bk_sb[:, jt:jt + 1], scalar2=None, op0=add,
                        )

                # bv broadcast [128, J] via K=1 f32 matmul of ones^T x bv_row
                pbv = ps01.tile([P, J], f32, tag="s")
                nc.tensor.matmul(pbv[:], ones_row[:], bv_row[:], start=True, stop=True)
                nc.vector.tensor_copy(bv_bc[:], pbv[:])

                for nt in range(NT):
                    pv = ps01.tile([P, J], f32, tag="s")
                    for kc in range(KC):
                        nc.tensor.matmul(
                            pv[:], xt_sb[:, kc, nt * P:(nt + 1) * P],
                            wv_sb[:, kc, :],
                            start=(kc == 0), stop=(kc == KC - 1),
                        )
                    nc.vector.tensor_tensor(
                        out=v_sb[:, nt, :], in0=pv[:], in1=bv_bc[:], op=add
                    )

            # ---- phase 2+3: attention per q-group ----
            with tc.tile_pool(name="alpha_u", bufs=3) as aup, \
                 tc.tile_pool(name="alpha_n", bufs=7) as anp, \
                 tc.tile_pool(name="alphaT", bufs=4) as atp, \
                 tc.tile_pool(name="sm", bufs=8) as smp, \
                 tc.tile_pool(name="outsb", bufs=2) as outp, \
                 tc.tile_pool(name="ps_a", bufs=1, space="PSUM") as ps_a, \
                 tc.tile_pool(name="ps_t", bufs=2, space="PSUM") as ps_t, \
                 tc.tile_pool(name="ps_z", bufs=2, space="PSUM") as ps_z:

                for g in range(NG):
                    for h in range(HL):
                        jt, jr = h // 2, (h % 2) * DK
                        an_tiles = []
                        for qq in range(GQ):
                            qi = g * GQ + qq
                            pa = ps_a.tile([P, N], f32)
                            for ch in range(NCH):
                                nc.tensor.matmul(
                                    pa[:, ch * 512:(ch + 1) * 512],
                                    qt_sb[jr:jr + DK, jt, qi * P:(qi + 1) * P],
                                    kt_sb[jr:jr + DK, jt, ch * 512:(ch + 1) * 512],
                                    start=True, stop=True,
                                )
                            ex = aup.tile([P, N], f32)
                            sums = smp.tile([P, 1], f32, tag="sums")
                            nc.scalar.activation(ex[:], pa[:], Exp,
                                                 bias=0.0, scale=0.125,
                                                 accum_out=sums[:])
                            rec = smp.tile([P, 1], f32, tag="rec")
                            nc.vector.reciprocal(rec[:], sums[:])
                            an = anp.tile([P, N], f32)
                            nc.vector.tensor_scalar(out=an[:], in0=ex[:],
                                                    scalar1=rec[:], scalar2=None,
                                                    op0=mult)
                            nc.sync.dma_start(
                                out=alpha_o[h, qi * P:(qi + 1) * P, :], in_=an[:]
                            )
                            an_tiles.append(an)

                        ztp = ps_z.tile([DK, 512], f32)
                        for kt in range(NT):
                            pt = ps_t.tile([P, 512], f32, tag="t")
                            for qq in range(GQ):
                                nc.tensor.transpose(
                                    pt[:, qq * P:(qq + 1) * P],
                                    an_tiles[qq][:, kt * P:(kt + 1) * P],
                                    ident[:],
                                )
                            at = atp.tile([P, 512], f32r)
                            if kt % 2 == 0:
                                nc.vector.tensor_copy(at[:], pt[:])
                            else:
                                nc.scalar.copy(at[:], pt[:])
                            nc.tensor.matmul(
                                ztp[:], v_sb[:, kt, h * DK:(h + 1) * DK], at[:],
                                start=(kt == 0), stop=(kt == NT - 1),
                            )
                        nc.vector.tensor_copy(
                            ztn_sb[jr:jr + DK, jt, g * 512:(g + 1) * 512], ztp[:]
                        )

                    # ---- phase 3: output projection for this q-group ----
                    for qq in range(GQ):
                        qi = g * GQ + qq
                        ob = outp.tile([P, D], f32)
                        for mc in range(2):
                            pp = ps_t.tile([P, 512], f32, tag="t")
                            for jt2 in range(2):
                                nc.tensor.matmul(
                                    pp[:], ztn_sb[:, jt2, qi * P:(qi + 1) * P],
                                    wo_sb[:, jt2, mc * 512:(mc + 1) * 512],
                                    start=(jt2 == 0), stop=(jt2 == 1),
                                )
                            nc.vector.tensor_copy(ob[:, mc * 512:(mc + 1) * 512], pp[:])
                        nc.sync.dma_start(out=po[qi * P:(qi + 1) * P, :], in_=ob[:])
                    nc.gpsimd.collective_compute(
                        "ReduceScatter", add,
                        replica_groups=[[0, 1, 2, 3], [4, 5, 6, 7]],
                        ins=[po[g * 512:(g + 1) * 512, :]], outs=[rs_tiles[g][:]],
                    )
                    nc.sync.dma_start(
                        out=out_o[g * P:(g + 1) * P, :], in_=rs_tiles[g][:]
                    )


    nc.compile()
    return nc


def _get_nc():
    if "nc" not in _CACHE:
        _CACHE["nc"] = _build()
    return _CACHE["nc"]


def _shard_inputs(X, Wq, bq, Wk, bk, Wv, bv, Wo, bo):
    in_maps = []
    for c in range(8):
        b, hg = c // 4, c % 4
        js = slice(hg * J, (hg + 1) * J)
        in_maps.append({
            "x": np.ascontiguousarray(X[b]),
            "wq": np.ascontiguousarray(Wq[:, js]),
            "wk": np.ascontiguousarray(Wk[:, js]),
            "wv": np.ascontiguousarray(Wv[:, js]),
            "wo": np.ascontiguousarray(Wo[js, :]),
            "bq": np.ascontiguousarray(bq[js]),
            "bk": np.ascontiguousarray(bk[js]),
            "bv": np.ascontiguousarray(bv[js]),
        })
    return in_maps


def _assemble(results, bo):
    out = np.empty((B, N, D), dtype=np.float32)
    alpha = np.empty((B, H, N, N), dtype=np.float32)
    for c in range(8):
        b, hg = c // 4, c % 4
        alpha[b, hg * HL:(hg + 1) * HL] = results[c]["alpha"]
        shard = results[c]["outp"]
        for g in range(NG):
            out[b, g * 512 + hg * P:g * 512 + (hg + 1) * P] = shard[g * P:(g + 1) * P]
    out += bo.astype(np.float32)
    return out, alpha


def _run(in_maps, trace=False):
    from concourse.bass_utils import run_bass_kernel_spmd
    nc = _get_nc()
    return run_bass_kernel_spmd(nc, in_maps, list(range(8)), trace=trace)


def kernel(X, Wq, bq, Wk, bk, Wv, bv, Wo, bo):
    args = [np.asarray(a, dtype=np.float32) for a in
            (X, Wq, bq, Wk, bk, Wv, bv, Wo, bo)]
    in_maps = _shard_inputs(*args)
    res = _run(in_maps, trace=False)
    return _assemble(res.results, args[-1])
